# revision 23
# baseline (speedup 1.0000x reference)
"""Bilinear sampler (flow warp) on 8 Trainium2 NeuronCores.

reference semantics (per batch b, f32):
  xn = 0.3*flow_x + linspace(-1,1,W)[j];  x = (xn + 1) * W/2
  yn = 0.3*flow_y + linspace(-1,1,H)[i];  y = (yn + 1) * H/2
  out = bilinear sample of image at (y, x). When x is outside [0, W-1)
  or y outside [0, H-1), the reference's clipped corner indices
  coincide and the weights cancel exactly, so out = 0 there.

Strategy: pure data parallel, 2 images per core.

The image is host-repacked into "entries" of 2 rows x 4 cols x 32 ch
(fp16, 512B): entry (par, hp, u) = rows (2*hp+par, 2*hp+par+1), cols
(2*u .. 2*u+3). For any corner (y0, x0) the entry
  idx = par*16384 + hp*128 + u,  par = y0&1, hp = y0>>1, u = x0>>1
contains the full 2x2 bilinear neighborhood; idx <= 32767 fits the
int16 index of gpsimd.dma_gather, and ONE 512B gather descriptor per
output pixel fetches all 4 corners. A DVE/ACT broadcast multiply with
an 8-slot weight vector (zeros at the two unused columns, and at
invalid pixels) plus three folds produces the f32 output.

Pixel layout: per-core pixel g = s*128 + p (p = partition). dma_gather
writes chunk i to partition i%128, so a tile of 16K pixels lands as
[128, K]; its int16 indices are consumed from [i%16, i//16], which the
kernel builds once into a rewrapped index tile via DMA+DVE shuffles.
"""

import numpy as np

import concourse.bacc as bacc
import concourse.mybir as mybir
import concourse.tile as tile
from concourse import library_config
from concourse.bass_utils import run_bass_kernel_spmd

B, H, W, C = 16, 256, 256, 32
NCORES = 8
BPC = B // NCORES            # images per core
NPIX = BPC * H * W           # pixels per core (131072)
P = 128                      # partitions
SLOTS = NPIX // P            # free slots per partition (1024)
K = 64                       # pixel slots per gather tile
NT = SLOTS // K              # gather tiles per core (8)
IMG_PIX = H * W              # pixels per image (65536)
NE = 2 * (H // 2) * (W // 2)  # entries per image (32768)
ES = 2 * 4 * C               # elems per entry (256 fp16 = 512B)

F32 = mybir.dt.float32
F16 = mybir.dt.float16
I32 = mybir.dt.int32
I16 = mybir.dt.int16
Alu = mybir.AluOpType


def _build_program(debug_taps=False):
    nc = bacc.Bacc("TRN2", target_bir_lowering=False, debug=False,
                   num_devices=NCORES)

    img = nc.dram_tensor("img", [BPC, NE, ES], F16, kind="ExternalInput").ap()
    flow = nc.dram_tensor("flow", [NPIX, 2], F32, kind="ExternalInput").ap()
    xbt = nc.dram_tensor("xbt", [P, SLOTS], F32, kind="ExternalInput").ap()
    ybt = nc.dram_tensor("ybt", [P, SLOTS], F32, kind="ExternalInput").ap()
    out = nc.dram_tensor("out", [NPIX, C], F32, kind="ExternalOutput").ap()
    if debug_taps:
        dIDXC = nc.dram_tensor("dIDXC", [P, SLOTS], I16,
                               kind="ExternalOutput").ap()
        dIDXG = nc.dram_tensor("dIDXG", [P, SLOTS * 8], I16,
                               kind="ExternalOutput").ap()
        dW8 = nc.dram_tensor("dW8", [P, SLOTS, 8], F16,
                             kind="ExternalOutput").ap()

    # flow/out are host-permuted to (p, s) order so these DMAs move
    # contiguous per-partition runs instead of 8B/128B scatters
    out_v = out.rearrange("(p s) c -> p s c", p=P)
    flow_v = flow.rearrange("(p s) c -> p s c", p=P)

    with tile.TileContext(nc) as tc:
        with (
            tc.tile_pool(name="keep", bufs=1) as kpool,
            tc.tile_pool(name="gather", bufs=2) as gpool,
            tc.tile_pool(name="outp", bufs=2) as opool,
        ):
            nc.gpsimd.load_library(library_config.mlp)

            W8 = kpool.tile([P, SLOTS, 8], F16, tag="W8")
            IDXG = kpool.tile([P, SLOTS * 8], I16, tag="IDXG")

            with (
                tc.tile_pool(name="tmp", bufs=1) as tpool,
                tc.tile_pool(name="tmp2", bufs=1) as t2pool,
            ):
                F = t2pool.tile([P, SLOTS, 2], F32, tag="F")
                XB = t2pool.tile([P, SLOTS], F32, tag="XB")
                YB = t2pool.tile([P, SLOTS], F32, tag="YB")
                nc.sync.dma_start(out=F[:], in_=flow_v)
                nc.sync.dma_start(out=XB[:], in_=xbt[:])
                nc.sync.dma_start(out=YB[:], in_=ybt[:])

                def t(tag):
                    return tpool.tile([P, SLOTS], F32, tag=tag, name=tag)

                ts, tt, cp = (nc.vector.tensor_scalar,
                              nc.vector.tensor_tensor,
                              nc.vector.tensor_copy)
                ats, att = nc.any.tensor_scalar, nc.any.tensor_tensor

                # pixel coords, replicating the reference's f32 rounding
                # order: xn = 0.3*f + xs[j]; x = (xn + 1) * 128 (exact *128)
                xp, yp = t("s0"), t("s1")
                ts(out=xp[:], in0=F[:, :, 0], scalar1=0.3, scalar2=None,
                   op0=Alu.mult)
                tt(out=xp[:], in0=xp[:], in1=XB[:], op=Alu.add)
                ts(out=xp[:], in0=xp[:], scalar1=1.0, scalar2=float(W) / 2.0,
                   op0=Alu.add, op1=Alu.mult)
                ts(out=yp[:], in0=F[:, :, 1], scalar1=0.3, scalar2=None,
                   op0=Alu.mult)
                tt(out=yp[:], in0=yp[:], in1=YB[:], op=Alu.add)
                ts(out=yp[:], in0=yp[:], scalar1=1.0, scalar2=float(H) / 2.0,
                   op0=Alu.add, op1=Alu.mult)

                # validity: 0 <= x < W-1 and 0 <= y < H-1 (else out = 0)
                v, vt = t("s2"), t("s3")
                ats(out=v[:], in0=xp[:], scalar1=0.0, scalar2=None,
                    op0=Alu.is_ge)
                ats(out=vt[:], in0=xp[:], scalar1=float(W - 1), scalar2=None,
                    op0=Alu.is_lt)
                att(out=v[:], in0=v[:], in1=vt[:], op=Alu.mult)
                ats(out=vt[:], in0=yp[:], scalar1=0.0, scalar2=None,
                    op0=Alu.is_ge)
                att(out=v[:], in0=v[:], in1=vt[:], op=Alu.mult)
                ats(out=vt[:], in0=yp[:], scalar1=float(H - 1), scalar2=None,
                    op0=Alu.is_lt)
                att(out=v[:], in0=v[:], in1=vt[:], op=Alu.mult)

                # corners: trunc(x) (== floor for x >= 0), clamp [0, W-2]
                xi = tpool.tile([P, SLOTS], I32, tag="xi", name="xi")
                yi = tpool.tile([P, SLOTS], I32, tag="yi", name="yi")
                cx, cy = t("s4"), t("s6")
                xh = t("s3")
                ts(out=xh[:], in0=xp[:], scalar1=-0.5, scalar2=None,
                   op0=Alu.add)
                cp(out=xi[:], in_=xh[:])
                cp(out=cx[:], in_=xi[:])
                ts(out=cx[:], in0=cx[:], scalar1=0.0, scalar2=float(W - 2),
                   op0=Alu.max, op1=Alu.min)
                yh = t("s3")
                ts(out=yh[:], in0=yp[:], scalar1=-0.5, scalar2=None,
                   op0=Alu.add)
                cp(out=yi[:], in_=yh[:])
                cp(out=cy[:], in_=yi[:])
                ts(out=cy[:], in0=cy[:], scalar1=0.0, scalar2=float(H - 2),
                   op0=Alu.max, op1=Alu.min)

                # fractions (exact in the valid range)
                fx, fy = t("s5"), t("s7")
                tt(out=fx[:], in0=xp[:], in1=cx[:], op=Alu.subtract)
                tt(out=fy[:], in0=yp[:], in1=cy[:], op=Alu.subtract)
                # xp, yp dead

                # entry decomposition: u = x0>>1, pm = x0&1 (same for y)
                uf, pm = t("s0"), t("s8")
                ts(out=uf[:], in0=cx[:], scalar1=0.5, scalar2=-0.25,
                   op0=Alu.mult, op1=Alu.add)
                cp(out=xi[:], in_=uf[:])
                cp(out=uf[:], in_=xi[:])
                ts(out=pm[:], in0=uf[:], scalar1=-2.0, scalar2=None,
                   op0=Alu.mult)
                tt(out=pm[:], in0=pm[:], in1=cx[:], op=Alu.add)
                # cx dead
                hpf, pr = t("s1"), t("s4")
                ts(out=hpf[:], in0=cy[:], scalar1=0.5, scalar2=-0.25,
                   op0=Alu.mult, op1=Alu.add)
                cp(out=yi[:], in_=hpf[:])
                cp(out=hpf[:], in_=yi[:])
                ts(out=pr[:], in0=hpf[:], scalar1=-2.0, scalar2=None,
                   op0=Alu.mult)
                tt(out=pr[:], in0=pr[:], in1=cy[:], op=Alu.add)
                # cy dead

                # entry index = pr*16384 + hpf*128 + u (<= 32767, f32-exact)
                idf = t("s6")
                ts(out=idf[:], in0=pr[:], scalar1=float(NE // 2), scalar2=None,
                   op0=Alu.mult)
                ts(out=hpf[:], in0=hpf[:], scalar1=float(W // 2), scalar2=None,
                   op0=Alu.mult)
                tt(out=idf[:], in0=idf[:], in1=hpf[:], op=Alu.add)
                tt(out=idf[:], in0=idf[:], in1=uf[:], op=Alu.add)
                IDXC = tpool.tile([P, SLOTS], I16, tag="IDXC", name="IDXC")
                cp(out=IDXC[:], in_=idf[:])
                # uf, hpf, pr, idf dead

                fx1, fy1 = t("s0"), t("s1")
                ts(out=fx1[:], in0=fx[:], scalar1=-1.0, scalar2=1.0,
                   op0=Alu.mult, op1=Alu.add)
                ts(out=fy1[:], in0=fy[:], scalar1=-1.0, scalar2=1.0,
                   op0=Alu.mult, op1=Alu.add)
                # fold validity into the y factors (zeroes all 4 weights)
                tt(out=fy[:], in0=fy[:], in1=v[:], op=Alu.mult)
                tt(out=fy1[:], in0=fy1[:], in1=v[:], op=Alu.mult)
                # v dead

                # 8-slot weight vector per pixel (slot = r*4 + c):
                #   row r=0: [wa*nm, wa*pm + wc*nm, wc*pm, 0]
                #   row r=1: [wb*nm, wb*pm + wd*nm, wd*pm, 0]
                nm = t("s2")
                ts(out=nm[:], in0=pm[:], scalar1=-1.0, scalar2=1.0,
                   op0=Alu.mult, op1=Alu.add)
                wa, wc = t("s4"), t("s6")
                att(out=wa[:], in0=fx1[:], in1=fy1[:], op=Alu.mult)
                att(out=wc[:], in0=fx[:], in1=fy1[:], op=Alu.mult)
                wb, wd = t("s3"), t("s9")
                att(out=wb[:], in0=fx1[:], in1=fy[:], op=Alu.mult)
                att(out=wd[:], in0=fx[:], in1=fy[:], op=Alu.mult)
                # fx, fy, fx1, fy1 dead
                t1, t2 = t("s5"), t("s7")
                att(out=W8[:, :, 0], in0=wa[:], in1=nm[:], op=Alu.mult)
                att(out=t1[:], in0=wa[:], in1=pm[:], op=Alu.mult)
                att(out=t2[:], in0=wc[:], in1=nm[:], op=Alu.mult)
                att(out=W8[:, :, 1], in0=t1[:], in1=t2[:], op=Alu.add)
                att(out=W8[:, :, 2], in0=wc[:], in1=pm[:], op=Alu.mult)
                att(out=W8[:, :, 4], in0=wb[:], in1=nm[:], op=Alu.mult)
                att(out=t1[:], in0=wb[:], in1=pm[:], op=Alu.mult)
                att(out=t2[:], in0=wd[:], in1=nm[:], op=Alu.mult)
                att(out=W8[:, :, 5], in0=t1[:], in1=t2[:], op=Alu.add)
                att(out=W8[:, :, 6], in0=wd[:], in1=pm[:], op=Alu.mult)

                # rewrap indices for dma_gather: chunk i reads its int16
                # index from [i%16, i//16] (replicated per 16-partition
                # group); i = s_local*128 + p  =>
                #   IDXG[16*rep + p%16, s*8 + p//16] = IDXC[p, s]
                SC = tpool.tile([16, SLOTS], I16, tag="SC", name="SC")
                for g in range(8):
                    nc.sync.dma_start(out=SC[:],
                                      in_=IDXC[16 * g:16 * (g + 1), :])
                    dst = IDXG[0:16, :].rearrange("p (s g) -> p s g", g=8)
                    nc.vector.tensor_copy(out=dst[:, :, g], in_=SC[:])
                for rep in range(1, 8):
                    nc.sync.dma_start(out=IDXG[16 * rep:16 * (rep + 1), :],
                                      in_=IDXG[0:16, :])

            for ti in range(NT):
                sl = slice(ti * K, (ti + 1) * K)
                G = gpool.tile([P, K, ES], F16, tag="G", name=f"G{ti}")
                nc.gpsimd.dma_gather(
                    G[:], img[ti // (NT // BPC)],
                    IDXG[:, ti * (K * 8):(ti + 1) * (K * 8)],
                    P * K, P * K, ES, single_packet=False,
                )
                G4 = G[:].rearrange("p k (x c) -> p k x c", c=C)
                # weight slots 3 and 7 are structurally zero; skip them
                w3 = W8[:, sl, 0:3].unsqueeze(3).broadcast_to([P, K, 3, C])
                nc.vector.tensor_tensor(out=G4[:, :, 0:3, :],
                                        in0=G4[:, :, 0:3, :], in1=w3,
                                        op=Alu.mult)
                w3b = W8[:, sl, 4:7].unsqueeze(3).broadcast_to([P, K, 3, C])
                nc.any.tensor_tensor(out=G4[:, :, 4:7, :],
                                     in0=G4[:, :, 4:7, :], in1=w3b,
                                     op=Alu.mult)
                nc.any.tensor_tensor(out=G4[:, :, 0:3, :],
                                     in0=G4[:, :, 0:3, :],
                                     in1=G4[:, :, 4:7, :], op=Alu.add)
                nc.any.tensor_tensor(out=G4[:, :, 0, :],
                                     in0=G4[:, :, 0, :],
                                     in1=G4[:, :, 1, :], op=Alu.add)
                O = opool.tile([P, K, C], F32, tag="O", name=f"O{ti}")
                nc.any.tensor_tensor(out=O[:], in0=G4[:, :, 0, :],
                                     in1=G4[:, :, 2, :], op=Alu.add)
                nc.sync.dma_start(out=out_v[:, sl, :], in_=O[:])

    nc.compile()
    return nc


_CACHED = {}


def _get_program(debug_taps=False):
    key = f"nc{debug_taps}"
    if key not in _CACHED:
        _CACHED[key] = _build_program(debug_taps)
    return _CACHED[key]


def _linspace_f32(n):
    # match jnp.linspace(-1, 1, n, dtype=float32): iota*step + start in f32
    step = np.float32(2.0) / np.float32(n - 1)
    return (np.arange(n, dtype=np.float32) * step + np.float32(-1.0)).astype(
        np.float32)


def _host_tables():
    # pixel (p, s): per-core pixel id g = s*128 + p; within-image id =
    # g % IMG_PIX; i = pid // W, j = pid % W
    pid = (np.arange(SLOTS)[None, :] * P + np.arange(P)[:, None]) % IMG_PIX
    i = pid // W
    j = pid % W
    xs = _linspace_f32(W)
    ys = _linspace_f32(H)
    return np.ascontiguousarray(xs[j]), np.ascontiguousarray(ys[i])


def _repack_images(image16):
    # image16: [nb, H, W, C] fp16 -> per image entries
    # [par, hp, u, r, c4, ch] = Ipad[2*hp + par + r, 2*u + c, ch]
    nb = image16.shape[0]
    pad = np.zeros((nb, H + 2, W + 2, C), np.float16)
    pad[:, :H, :W, :] = image16
    sb, sh, sw, sc = pad.strides
    ev = np.lib.stride_tricks.as_strided(
        pad,
        shape=(nb, 2, H // 2, W // 2, 2, 4, C),
        strides=(sb, sh, 2 * sh, 2 * sw, sh, sw, sc),
    )
    return np.ascontiguousarray(ev).reshape(nb, NE, ES)


def kernel(image: np.ndarray, flow: np.ndarray) -> np.ndarray:
    image = np.asarray(image)
    flow = np.asarray(flow)
    assert image.shape == (B, H, W, C) and flow.shape == (B, H, W, 2)

    nc = _get_program()
    xbt, ybt = _host_tables()

    entries = _repack_images(image.astype(np.float16))  # [B, NE, ES]
    flow32 = np.ascontiguousarray(flow, dtype=np.float32).reshape(
        NCORES, SLOTS, P, 2).transpose(0, 2, 1, 3).reshape(NCORES, NPIX, 2)

    in_maps = []
    for c in range(NCORES):
        in_maps.append({
            "img": np.ascontiguousarray(entries[BPC * c:BPC * (c + 1)]),
            "flow": np.ascontiguousarray(flow32[c]),
            "xbt": xbt,
            "ybt": ybt,
        })

    res = run_bass_kernel_spmd(nc, in_maps, list(range(NCORES)))
    _CACHED["last_result"] = res
    outs = [res.results[c]["out"].reshape(P, SLOTS, C).transpose(1, 0, 2)
            .reshape(NPIX, C) for c in range(NCORES)]
    return np.concatenate(outs, axis=0).reshape(B, H, W, C).astype(np.float32)


# revision 24
# speedup vs baseline: 1.1881x; 1.1881x over previous
"""Bilinear sampler (flow warp) on 8 Trainium2 NeuronCores.

reference semantics (per batch b, f32):
  xn = 0.3*flow_x + linspace(-1,1,W)[j];  x = (xn + 1) * W/2
  yn = 0.3*flow_y + linspace(-1,1,H)[i];  y = (yn + 1) * H/2
  out = bilinear sample of image at (y, x). When x is outside [0, W-1)
  or y outside [0, H-1), the reference's clipped corner indices
  coincide and the weights cancel exactly, so out = 0 there.

Strategy: pure data parallel, 2 images per core.

The image is host-repacked into "entries" of 2 rows x 4 cols x 32 ch
(fp16, 512B): entry (par, hp, u) = rows (2*hp+par, 2*hp+par+1), cols
(2*u .. 2*u+3). For any corner (y0, x0) the entry
  idx = par*16384 + hp*128 + u,  par = y0&1, hp = y0>>1, u = x0>>1
contains the full 2x2 bilinear neighborhood; idx <= 32767 fits the
int16 index of gpsimd.dma_gather, and ONE 512B gather descriptor per
output pixel fetches all 4 corners. A DVE/ACT broadcast multiply with
an 8-slot weight vector (zeros at the two unused columns, and at
invalid pixels) plus three folds produces the f32 output.

Pixel layout: per-core pixel g = s*128 + p (p = partition). dma_gather
writes chunk i to partition i%128, so a tile of 16K pixels lands as
[128, K]; its int16 indices are consumed from [i%16, i//16], which the
kernel builds once into a rewrapped index tile via DMA+DVE shuffles.
"""

import numpy as np

import concourse.bacc as bacc
import concourse.mybir as mybir
import concourse.tile as tile
from concourse import library_config
from concourse.bass_utils import run_bass_kernel_spmd

B, H, W, C = 16, 256, 256, 32
NCORES = 8
BPC = B // NCORES            # images per core
NPIX = BPC * H * W           # pixels per core (131072)
P = 128                      # partitions
SLOTS = NPIX // P            # free slots per partition (1024)
K = 32                       # pixel slots per gather tile
NT = SLOTS // K              # gather tiles per core (8)
IMG_PIX = H * W              # pixels per image (65536)
NE = 2 * (H // 2) * (W // 2)  # entries per image (32768)
ES = 2 * 4 * C               # elems per entry (256 fp16 = 512B)

F32 = mybir.dt.float32
F16 = mybir.dt.float16
I32 = mybir.dt.int32
I16 = mybir.dt.int16
Alu = mybir.AluOpType


def _build_program(debug_taps=False):
    nc = bacc.Bacc("TRN2", target_bir_lowering=False, debug=False,
                   num_devices=NCORES)

    img = nc.dram_tensor("img", [BPC, NE, ES], F16, kind="ExternalInput").ap()
    flow = nc.dram_tensor("flow", [NPIX, 2], F32, kind="ExternalInput").ap()
    xbt = nc.dram_tensor("xbt", [P, SLOTS], F32, kind="ExternalInput").ap()
    ybt = nc.dram_tensor("ybt", [P, SLOTS], F32, kind="ExternalInput").ap()
    out = nc.dram_tensor("out", [NPIX, C], F32, kind="ExternalOutput").ap()
    if debug_taps:
        dIDXC = nc.dram_tensor("dIDXC", [P, SLOTS], I16,
                               kind="ExternalOutput").ap()
        dIDXG = nc.dram_tensor("dIDXG", [P, SLOTS * 8], I16,
                               kind="ExternalOutput").ap()
        dW8 = nc.dram_tensor("dW8", [P, SLOTS, 8], F16,
                             kind="ExternalOutput").ap()

    # flow/out are host-permuted to (p, s) order so these DMAs move
    # contiguous per-partition runs instead of 8B/128B scatters
    out_v = out.rearrange("(p s) c -> p s c", p=P)
    flow_v = flow.rearrange("(p s) c -> p s c", p=P)

    with tile.TileContext(nc) as tc:
        with (
            tc.tile_pool(name="keep", bufs=1) as kpool,
            tc.tile_pool(name="gather", bufs=3) as gpool,
            tc.tile_pool(name="outp", bufs=3) as opool,
        ):
            nc.gpsimd.load_library(library_config.mlp)

            W8 = kpool.tile([P, SLOTS, 8], F16, tag="W8")
            IDXG = kpool.tile([P, SLOTS * 8], I16, tag="IDXG")

            with (
                tc.tile_pool(name="tmp", bufs=1) as tpool,
                tc.tile_pool(name="tmp2", bufs=1) as t2pool,
            ):
                F = t2pool.tile([P, SLOTS, 2], F32, tag="F")
                XB = t2pool.tile([P, SLOTS], F32, tag="XB")
                YB = t2pool.tile([P, SLOTS], F32, tag="YB")
                nc.sync.dma_start(out=F[:], in_=flow_v)
                nc.sync.dma_start(out=XB[:], in_=xbt[:])
                nc.sync.dma_start(out=YB[:], in_=ybt[:])

                def t(tag):
                    return tpool.tile([P, SLOTS], F32, tag=tag, name=tag)

                ts, tt, cp = (nc.vector.tensor_scalar,
                              nc.vector.tensor_tensor,
                              nc.vector.tensor_copy)
                ats, att = nc.any.tensor_scalar, nc.any.tensor_tensor

                # pixel coords, replicating the reference's f32 rounding
                # order: xn = 0.3*f + xs[j]; x = (xn + 1) * 128 (exact *128)
                xp, yp = t("s0"), t("s1")
                ts(out=xp[:], in0=F[:, :, 0], scalar1=0.3, scalar2=None,
                   op0=Alu.mult)
                tt(out=xp[:], in0=xp[:], in1=XB[:], op=Alu.add)
                ts(out=xp[:], in0=xp[:], scalar1=1.0, scalar2=float(W) / 2.0,
                   op0=Alu.add, op1=Alu.mult)
                ts(out=yp[:], in0=F[:, :, 1], scalar1=0.3, scalar2=None,
                   op0=Alu.mult)
                tt(out=yp[:], in0=yp[:], in1=YB[:], op=Alu.add)
                ts(out=yp[:], in0=yp[:], scalar1=1.0, scalar2=float(H) / 2.0,
                   op0=Alu.add, op1=Alu.mult)

                # validity: 0 <= x < W-1 and 0 <= y < H-1 (else out = 0)
                v, vt = t("s2"), t("s3")
                ats(out=v[:], in0=xp[:], scalar1=0.0, scalar2=None,
                    op0=Alu.is_ge)
                ats(out=vt[:], in0=xp[:], scalar1=float(W - 1), scalar2=None,
                    op0=Alu.is_lt)
                att(out=v[:], in0=v[:], in1=vt[:], op=Alu.mult)
                ats(out=vt[:], in0=yp[:], scalar1=0.0, scalar2=None,
                    op0=Alu.is_ge)
                att(out=v[:], in0=v[:], in1=vt[:], op=Alu.mult)
                ats(out=vt[:], in0=yp[:], scalar1=float(H - 1), scalar2=None,
                    op0=Alu.is_lt)
                att(out=v[:], in0=v[:], in1=vt[:], op=Alu.mult)

                # corners: trunc(x) (== floor for x >= 0), clamp [0, W-2]
                xi = tpool.tile([P, SLOTS], I32, tag="xi", name="xi")
                yi = tpool.tile([P, SLOTS], I32, tag="yi", name="yi")
                cx, cy = t("s4"), t("s6")
                xh = t("s3")
                ts(out=xh[:], in0=xp[:], scalar1=-0.5, scalar2=None,
                   op0=Alu.add)
                cp(out=xi[:], in_=xh[:])
                cp(out=cx[:], in_=xi[:])
                ts(out=cx[:], in0=cx[:], scalar1=0.0, scalar2=float(W - 2),
                   op0=Alu.max, op1=Alu.min)
                yh = t("s3")
                ts(out=yh[:], in0=yp[:], scalar1=-0.5, scalar2=None,
                   op0=Alu.add)
                cp(out=yi[:], in_=yh[:])
                cp(out=cy[:], in_=yi[:])
                ts(out=cy[:], in0=cy[:], scalar1=0.0, scalar2=float(H - 2),
                   op0=Alu.max, op1=Alu.min)

                # fractions (exact in the valid range)
                fx, fy = t("s5"), t("s7")
                tt(out=fx[:], in0=xp[:], in1=cx[:], op=Alu.subtract)
                tt(out=fy[:], in0=yp[:], in1=cy[:], op=Alu.subtract)
                # xp, yp dead

                # entry decomposition: u = x0>>1, pm = x0&1 (same for y)
                uf, pm = t("s0"), t("s8")
                ts(out=uf[:], in0=cx[:], scalar1=0.5, scalar2=-0.25,
                   op0=Alu.mult, op1=Alu.add)
                cp(out=xi[:], in_=uf[:])
                cp(out=uf[:], in_=xi[:])
                ts(out=pm[:], in0=uf[:], scalar1=-2.0, scalar2=None,
                   op0=Alu.mult)
                tt(out=pm[:], in0=pm[:], in1=cx[:], op=Alu.add)
                # cx dead
                hpf, pr = t("s1"), t("s4")
                ts(out=hpf[:], in0=cy[:], scalar1=0.5, scalar2=-0.25,
                   op0=Alu.mult, op1=Alu.add)
                cp(out=yi[:], in_=hpf[:])
                cp(out=hpf[:], in_=yi[:])
                ts(out=pr[:], in0=hpf[:], scalar1=-2.0, scalar2=None,
                   op0=Alu.mult)
                tt(out=pr[:], in0=pr[:], in1=cy[:], op=Alu.add)
                # cy dead

                # entry index = pr*16384 + hpf*128 + u (<= 32767, f32-exact)
                idf = t("s6")
                ts(out=idf[:], in0=pr[:], scalar1=float(NE // 2), scalar2=None,
                   op0=Alu.mult)
                ts(out=hpf[:], in0=hpf[:], scalar1=float(W // 2), scalar2=None,
                   op0=Alu.mult)
                tt(out=idf[:], in0=idf[:], in1=hpf[:], op=Alu.add)
                tt(out=idf[:], in0=idf[:], in1=uf[:], op=Alu.add)
                IDXC = tpool.tile([P, SLOTS], I16, tag="IDXC", name="IDXC")
                cp(out=IDXC[:], in_=idf[:])
                # uf, hpf, pr, idf dead

                fx1, fy1 = t("s0"), t("s1")
                ts(out=fx1[:], in0=fx[:], scalar1=-1.0, scalar2=1.0,
                   op0=Alu.mult, op1=Alu.add)
                ts(out=fy1[:], in0=fy[:], scalar1=-1.0, scalar2=1.0,
                   op0=Alu.mult, op1=Alu.add)
                # fold validity into the y factors (zeroes all 4 weights)
                tt(out=fy[:], in0=fy[:], in1=v[:], op=Alu.mult)
                tt(out=fy1[:], in0=fy1[:], in1=v[:], op=Alu.mult)
                # v dead

                # 8-slot weight vector per pixel (slot = r*4 + c):
                #   row r=0: [wa*nm, wa*pm + wc*nm, wc*pm, 0]
                #   row r=1: [wb*nm, wb*pm + wd*nm, wd*pm, 0]
                nm = t("s2")
                ts(out=nm[:], in0=pm[:], scalar1=-1.0, scalar2=1.0,
                   op0=Alu.mult, op1=Alu.add)
                wa, wc = t("s4"), t("s6")
                att(out=wa[:], in0=fx1[:], in1=fy1[:], op=Alu.mult)
                att(out=wc[:], in0=fx[:], in1=fy1[:], op=Alu.mult)
                wb, wd = t("s3"), t("s9")
                att(out=wb[:], in0=fx1[:], in1=fy[:], op=Alu.mult)
                att(out=wd[:], in0=fx[:], in1=fy[:], op=Alu.mult)
                # fx, fy, fx1, fy1 dead
                t1, t2 = t("s5"), t("s7")
                att(out=W8[:, :, 0], in0=wa[:], in1=nm[:], op=Alu.mult)
                att(out=t1[:], in0=wa[:], in1=pm[:], op=Alu.mult)
                att(out=t2[:], in0=wc[:], in1=nm[:], op=Alu.mult)
                att(out=W8[:, :, 1], in0=t1[:], in1=t2[:], op=Alu.add)
                att(out=W8[:, :, 2], in0=wc[:], in1=pm[:], op=Alu.mult)
                att(out=W8[:, :, 4], in0=wb[:], in1=nm[:], op=Alu.mult)
                att(out=t1[:], in0=wb[:], in1=pm[:], op=Alu.mult)
                att(out=t2[:], in0=wd[:], in1=nm[:], op=Alu.mult)
                att(out=W8[:, :, 5], in0=t1[:], in1=t2[:], op=Alu.add)
                att(out=W8[:, :, 6], in0=wd[:], in1=pm[:], op=Alu.mult)

                # rewrap indices for dma_gather: chunk i reads its int16
                # index from [i%16, i//16] (replicated per 16-partition
                # group); i = s_local*128 + p  =>
                #   IDXG[16*rep + p%16, s*8 + p//16] = IDXC[p, s]
                SC = tpool.tile([16, SLOTS], I16, tag="SC", name="SC")
                for g in range(8):
                    nc.sync.dma_start(out=SC[:],
                                      in_=IDXC[16 * g:16 * (g + 1), :])
                    dst = IDXG[0:16, :].rearrange("p (s g) -> p s g", g=8)
                    nc.vector.tensor_copy(out=dst[:, :, g], in_=SC[:])
                for rep in range(1, 8):
                    nc.sync.dma_start(out=IDXG[16 * rep:16 * (rep + 1), :],
                                      in_=IDXG[0:16, :])

            for ti in range(NT):
                sl = slice(ti * K, (ti + 1) * K)
                G = gpool.tile([P, K, ES], F16, tag="G", name=f"G{ti}")
                nc.gpsimd.dma_gather(
                    G[:], img[ti // (NT // BPC)],
                    IDXG[:, ti * (K * 8):(ti + 1) * (K * 8)],
                    P * K, P * K, ES, single_packet=False,
                )
                G4 = G[:].rearrange("p k (x c) -> p k x c", c=C)
                # weight slots 3 and 7 are structurally zero; skip them
                w3 = W8[:, sl, 0:3].unsqueeze(3).broadcast_to([P, K, 3, C])
                nc.vector.tensor_tensor(out=G4[:, :, 0:3, :],
                                        in0=G4[:, :, 0:3, :], in1=w3,
                                        op=Alu.mult)
                w3b = W8[:, sl, 4:7].unsqueeze(3).broadcast_to([P, K, 3, C])
                nc.any.tensor_tensor(out=G4[:, :, 4:7, :],
                                     in0=G4[:, :, 4:7, :], in1=w3b,
                                     op=Alu.mult)
                nc.any.tensor_tensor(out=G4[:, :, 0:3, :],
                                     in0=G4[:, :, 0:3, :],
                                     in1=G4[:, :, 4:7, :], op=Alu.add)
                nc.any.tensor_tensor(out=G4[:, :, 0, :],
                                     in0=G4[:, :, 0, :],
                                     in1=G4[:, :, 1, :], op=Alu.add)
                O = opool.tile([P, K, C], F32, tag="O", name=f"O{ti}")
                nc.any.tensor_tensor(out=O[:], in0=G4[:, :, 0, :],
                                     in1=G4[:, :, 2, :], op=Alu.add)
                nc.sync.dma_start(out=out_v[:, sl, :], in_=O[:])

    nc.compile()
    return nc


_CACHED = {}


def _get_program(debug_taps=False):
    key = f"nc{debug_taps}"
    if key not in _CACHED:
        _CACHED[key] = _build_program(debug_taps)
    return _CACHED[key]


def _linspace_f32(n):
    # match jnp.linspace(-1, 1, n, dtype=float32): iota*step + start in f32
    step = np.float32(2.0) / np.float32(n - 1)
    return (np.arange(n, dtype=np.float32) * step + np.float32(-1.0)).astype(
        np.float32)


def _host_tables():
    # pixel (p, s): per-core pixel id g = s*128 + p; within-image id =
    # g % IMG_PIX; i = pid // W, j = pid % W
    pid = (np.arange(SLOTS)[None, :] * P + np.arange(P)[:, None]) % IMG_PIX
    i = pid // W
    j = pid % W
    xs = _linspace_f32(W)
    ys = _linspace_f32(H)
    return np.ascontiguousarray(xs[j]), np.ascontiguousarray(ys[i])


def _repack_images(image16):
    # image16: [nb, H, W, C] fp16 -> per image entries
    # [par, hp, u, r, c4, ch] = Ipad[2*hp + par + r, 2*u + c, ch]
    nb = image16.shape[0]
    pad = np.zeros((nb, H + 2, W + 2, C), np.float16)
    pad[:, :H, :W, :] = image16
    sb, sh, sw, sc = pad.strides
    ev = np.lib.stride_tricks.as_strided(
        pad,
        shape=(nb, 2, H // 2, W // 2, 2, 4, C),
        strides=(sb, sh, 2 * sh, 2 * sw, sh, sw, sc),
    )
    return np.ascontiguousarray(ev).reshape(nb, NE, ES)


def kernel(image: np.ndarray, flow: np.ndarray) -> np.ndarray:
    image = np.asarray(image)
    flow = np.asarray(flow)
    assert image.shape == (B, H, W, C) and flow.shape == (B, H, W, 2)

    nc = _get_program()
    xbt, ybt = _host_tables()

    entries = _repack_images(image.astype(np.float16))  # [B, NE, ES]
    flow32 = np.ascontiguousarray(flow, dtype=np.float32).reshape(
        NCORES, SLOTS, P, 2).transpose(0, 2, 1, 3).reshape(NCORES, NPIX, 2)

    in_maps = []
    for c in range(NCORES):
        in_maps.append({
            "img": np.ascontiguousarray(entries[BPC * c:BPC * (c + 1)]),
            "flow": np.ascontiguousarray(flow32[c]),
            "xbt": xbt,
            "ybt": ybt,
        })

    res = run_bass_kernel_spmd(nc, in_maps, list(range(NCORES)))
    _CACHED["last_result"] = res
    outs = [res.results[c]["out"].reshape(P, SLOTS, C).transpose(1, 0, 2)
            .reshape(NPIX, C) for c in range(NCORES)]
    return np.concatenate(outs, axis=0).reshape(B, H, W, C).astype(np.float32)
